# revision 15
# baseline (speedup 1.0000x reference)
"""MoE feed-forward (top-1 routing, capacity 640, swiglu experts) on 8 trn2 cores.

Strategy (expert-parallel, per the sharding hint):
  * Host: router matmul/softmax/argmax + capacity-slot assignment (index
    plumbing, ~0.1% of FLOPs), gathers tokens per expert, pairs a heavy
    expert with a light one per core, 2 experts per core.
  * Device (Bass/Tile, per core): grouped GEMM  h = x @ W1  -> swiglu ->
    y = g @ W2.  Matmuls in bf16 with fp32 accumulate.  Tokens are the
    MOVING dim in both GEMMs (GEMM1 emits hT [feat, tok]; GEMM2 emits
    yT [d, tok]), so slot sizes are token-granular (no 128-row padding)
    and every matmul is >=212 columns wide, keeping LDWEIGHTS hidden.
    W1 streams per feature tile on the sync queue; W2 streams per k-tile
    on the vector queue, paced inside the GEMM1 loop; x/b1 load upfront.
  * Host: scatter expert outputs back to token order, applying the
    combine gates (and b2 / dense fallback when nonzero) on the fly.
"""

import os
import sys

import numpy as np


def _ensure_concourse():
    try:
        import concourse.bass  # noqa: F401
    except Exception:
        for p in ("/opt/trn_rl_repo", "/root/.axon_site/_ro/trn_rl_repo"):
            if os.path.isdir(p) and p not in sys.path:
                sys.path.insert(0, p)
        import concourse.bass  # noqa: F401


# Problem constants (hardcoded per the task contract).
B, S, D, H, E = 4, 2048, 768, 3072, 16
N = B * S
C = 640  # capacity per expert (ceil(1.25 * N / E))
FALLBACK_W = 1.0
NCORES = 8
EL = E // NCORES  # experts per core = 2
KD = D // 128  # 6 k-tiles for GEMM1 contraction
FB = (2 * H) // 128  # 48 feature blocks of GEMM1 output
FP = FB // 2  # 24 swiglu pairs == k-tiles of GEMM2 contraction
KH = H // 128  # 24

_NC_CACHE = {}  # (T0, T1) -> compiled Bass program
_WCACHE = {}  # weight reorder cache
LAST = None  # BassKernelResults of the most recent run (for profiling)


def _chunks(T):
    """Split T tokens into <=512 moving-dim chunks, all >=212 when possible."""
    n = max(1, -(-T // 512))
    base = T // n
    out, off = [], 0
    for i in range(n):
        w = base + (1 if i < T - base * n else 0)
        out.append((off, w))
        off += w
    return out


def _build_nc(ts):
    """Per-core Bass program: 2 expert slots with ts[s] tokens each."""
    import concourse.bacc as bacc
    import concourse.mybir as mybir
    import concourse.tile as tile
    from contextlib import ExitStack

    f32 = mybir.dt.float32
    bf16 = mybir.dt.bfloat16
    AF = mybir.ActivationFunctionType
    ALU = mybir.AluOpType

    tot = sum(ts)

    nc = bacc.Bacc("TRN2", target_bir_lowering=False)
    # Host-side layouts are pre-tiled so every DMA is 2D [128, contiguous].
    xt = nc.dram_tensor("xt", [128, KD * tot], bf16, kind="ExternalInput")
    w1r = nc.dram_tensor("w1r", [EL, FP, 128, 2 * KD * 128], bf16, kind="ExternalInput")
    w2t = nc.dram_tensor("w2t", [EL, 128, KH * D], bf16, kind="ExternalInput")
    b1t = nc.dram_tensor("b1t", [EL, 128, FB], f32, kind="ExternalInput")
    y = nc.dram_tensor("y", [D, tot], f32, kind="ExternalOutput")

    with tile.TileContext(nc) as tc, ExitStack() as ctx:
        xp = ctx.enter_context(tc.tile_pool(name="xp", bufs=1))
        gp = ctx.enter_context(tc.tile_pool(name="gp", bufs=1))
        w2p = ctx.enter_context(tc.tile_pool(name="w2p", bufs=2))
        w1p = ctx.enter_context(tc.tile_pool(name="w1p", bufs=6))
        sap = ctx.enter_context(tc.tile_pool(name="sap", bufs=3))
        cst = ctx.enter_context(tc.tile_pool(name="cst", bufs=2))
        yp = ctx.enter_context(tc.tile_pool(name="yp", bufs=4))
        p1 = ctx.enter_context(tc.tile_pool(name="p1", bufs=3, space="PSUM"))
        p2 = ctx.enter_context(tc.tile_pool(name="p2", bufs=2, space="PSUM"))
        # Tensor-engine clock warmup: the PE DVFS ramps with sustained use,
        # so spin throwaway matmuls on a scratch tile while the first real
        # x/W1 transfers are still in flight.
        wz = xp.tile([128, 256], bf16, tag="wz")
        nc.vector.memset(wz[:], 0.0)
        for _ in range(30):
            pwt = p2.tile([128, 256], f32, tag="p2")
            nc.tensor.matmul(pwt[:], lhsT=wz[:, :128], rhs=wz[:], start=True, stop=True)

        # All x and b1 loads upfront.  x goes through the two hardware-DGE
        # queues (sync/scalar) whose descriptor generation is ~free; gpsimd
        # (software DGE, ~650ns/descriptor) only carries the tiny b1 loads.
        # Slot0's even k-chunks are emitted on sync *around* the first W1
        # tile (inside the fp loop) so the first matmul starts early.
        xsb = xp.tile([128, KD * tot], bf16, tag="x")

        def x_dma(eng, e, k):
            T = ts[e]
            xoff = KD * ts[0] if e else 0
            eng.dma_start(
                xsb[:, xoff + k * T : xoff + (k + 1) * T],
                xt[:, xoff + k * T : xoff + (k + 1) * T],
            )

        b1sb0 = cst.tile([128, FB], f32, tag="b1_0")
        b1sb1 = cst.tile([128, FB], f32, tag="b1_1")
        b1sb = [b1sb0, b1sb1]
        x_dma(nc.sync, 0, 0)
        for k in (1, 3, 5):
            x_dma(nc.scalar, 0, k)
        # slot1 b1 is not needed until ~half-way; keep it off the hot queues
        nc.gpsimd.dma_start(b1sb[1][:], b1t[1, :, :])

        gt = gp.tile([128, KH * tot], bf16, tag="g")

        for e in range(EL):
            T = ts[e]
            xoff = KD * ts[0] if e else 0
            goff = KH * ts[0] if e else 0
            yoff = ts[0] if e else 0
            chs = _chunks(T)

            # W2 for this expert: one SBUF tile, streamed as per-k chunks
            # from the vector queue, paced by the GEMM1 fp loop below.
            w2sb = w2p.tile([128, KH * D], bf16, tag="w2")

            # GEMM1 + swiglu: hT tiles [feat 128, tok chunk]
            for fp in range(FP):
                w1t = w1p.tile([128, 2 * KD * 128], bf16, tag="w1")
                if e == 0 and fp == 0:
                    # split so the first matmuls wait only on the a-half,
                    # and slot the remaining even x chunks behind it
                    nc.sync.dma_start(w1t[:, : KD * 128], w1r[e, fp, :, : KD * 128])
                    x_dma(nc.sync, 0, 2)
                    x_dma(nc.sync, 0, 4)
                    nc.sync.dma_start(w1t[:, KD * 128 :], w1r[e, fp, :, KD * 128 :])
                    nc.sync.dma_start(b1sb[0][:], b1t[0, :, :])
                else:
                    nc.sync.dma_start(w1t[:], w1r[e, fp, :, :])
                w1a = w1t[:, : KD * 128]
                w1b = w1t[:, KD * 128 :]
                for coff, cw in chs:
                    pa = p1.tile([128, cw], f32, tag="pa")
                    pb = p1.tile([128, cw], f32, tag="pb")
                    for k in range(KD):
                        nc.tensor.matmul(
                            pa[:],
                            lhsT=w1a[:, k * 128 : (k + 1) * 128],
                            rhs=xsb[:, xoff + k * T + coff : xoff + k * T + coff + cw],
                            start=(k == 0),
                            stop=(k == KD - 1),
                        )
                    for k in range(KD):
                        nc.tensor.matmul(
                            pb[:],
                            lhsT=w1b[:, k * 128 : (k + 1) * 128],
                            rhs=xsb[:, xoff + k * T + coff : xoff + k * T + coff + cw],
                            start=(k == 0),
                            stop=(k == KD - 1),
                        )
                    sa = sap.tile([128, cw], f32, tag="sa")
                    # silu(a + b1_a)
                    nc.scalar.activation(
                        sa[:], pa[:], AF.Silu, bias=b1sb[e][:, fp : fp + 1], scale=1.0
                    )
                    # g = (b + b1_b) * silu(...)
                    nc.vector.scalar_tensor_tensor(
                        out=gt[:, goff + fp * T + coff : goff + fp * T + coff + cw],
                        in0=pb[:],
                        scalar=b1sb[e][:, FP + fp : FP + fp + 1],
                        in1=sa[:],
                        op0=ALU.add,
                        op1=ALU.mult,
                    )
                # deferred slot1 x loads, off the startup bandwidth peak
                if e == 0 and 6 <= fp < 6 + KD:
                    x_dma(nc.scalar, 1, fp - 6)
                # paced W2 k-pair load (scalar queue, after this fp's math)
                if fp % 2 == 0 and fp < KH:
                    nc.scalar.dma_start(
                        w2sb[:, fp * D : (fp + 2) * D], w2t[e, :, fp * D : (fp + 2) * D]
                    )

            # GEMM2: yT[d 128, tok chunk] = sum_k W2[h_k, d].T @ g[h_k, tok]
            for coff, cw in chs:
                for dt in range(KD):
                    pt = p2.tile([128, cw], f32, tag="p2")
                    for k in range(KH):
                        nc.tensor.matmul(
                            pt[:],
                            lhsT=w2sb[:, k * D + dt * 128 : k * D + (dt + 1) * 128],
                            rhs=gt[:, goff + k * T + coff : goff + k * T + coff + cw],
                            start=(k == 0),
                            stop=(k == KH - 1),
                        )
                    ysb = yp.tile([128, cw], f32, tag="y")
                    nc.scalar.activation(ysb[:], pt[:], AF.Copy, bias=0.0, scale=1.0)
                    nc.scalar.dma_start(
                        y[dt * 128 : (dt + 1) * 128, yoff + coff : yoff + coff + cw],
                        ysb[:],
                    )
    nc.compile()
    return nc


def _get_nc(ts):
    nc = _NC_CACHE.get(ts)
    if nc is None:
        nc = _NC_CACHE[ts] = _build_nc(ts)
    return nc


def _reorder_weights(W1, W2, b1):
    key = (W1.__array_interface__["data"][0], W2.__array_interface__["data"][0])
    hit = _WCACHE.get(key)
    if hit is not None:
        return hit
    import ml_dtypes

    W1 = np.ascontiguousarray(W1, dtype=np.float32)
    W2 = np.ascontiguousarray(W2, dtype=np.float32)
    b1 = np.ascontiguousarray(b1, dtype=np.float32)
    # W1 [E, D, 2H] -> [E, FB, 128p(d within k), KD*128(f)]
    w1f = (
        W1.reshape(E, KD, 128, FB, 128)
        .transpose(0, 3, 2, 1, 4)
        .reshape(E, FB, 128, KD * 128)
        .astype(ml_dtypes.bfloat16)
    )
    # combine swiglu pair (fp, fp+FP) into one contiguous block per DMA
    w1r = np.ascontiguousarray(np.concatenate([w1f[:, :FP], w1f[:, FP:]], axis=-1))
    # W2 [E, H, D] -> [E, 128p(h within k), KH*D]
    w2t = np.ascontiguousarray(
        W2.reshape(E, KH, 128, D)
        .transpose(0, 2, 1, 3)
        .reshape(E, 128, KH * D)
        .astype(ml_dtypes.bfloat16)
    )
    # b1 [E, 2H] -> [E, 128, FB]
    b1t = np.ascontiguousarray(b1.reshape(E, FB, 128).transpose(0, 2, 1))
    out = (w1r, w2t, b1t)
    _WCACHE.clear()
    _WCACHE[key] = out
    return out


def _route(x_flat, Wr):
    logits = x_flat @ np.ascontiguousarray(Wr, dtype=np.float32)  # [N, E]
    lmax = logits.max(axis=-1, keepdims=True)
    p = np.exp(logits - lmax)
    gates = p / p.sum(axis=-1, keepdims=True)
    expert = np.argmax(gates, axis=-1)
    # slot = occurrence index of each token within its expert's queue
    order = np.argsort(expert, kind="stable")
    sorted_e = expert[order]
    starts = np.searchsorted(sorted_e, np.arange(E))
    within = np.arange(N) - starts[sorted_e]
    slot = np.empty(N, np.int64)
    slot[order] = within
    kept = slot < C
    top_idx = np.zeros((C, E), np.int32)
    valid = np.zeros((C, E), np.float32)
    tok = np.arange(N, dtype=np.int32)
    top_idx[slot[kept], expert[kept]] = tok[kept]
    valid[slot[kept], expert[kept]] = 1.0
    w_ce = gates[top_idx, np.arange(E)[None, :]].astype(np.float32) * valid  # [C, E]
    n_kept = np.minimum(np.bincount(expert, minlength=E), C)  # [E]
    return gates, expert, kept, top_idx, valid, w_ce, n_kept


def _r16(n):
    return max(16, (int(n) + 3) & ~3)


def kernel(x, Wr, W1, b1, W2, b2, W1f, b1f, W2f, b2f, _trace=False):
    global LAST
    _ensure_concourse()
    import ml_dtypes
    from concourse.bass_utils import run_bass_kernel_spmd

    x_flat = np.ascontiguousarray(np.asarray(x).reshape(N, D), dtype=np.float32)
    gates, expert, kept, top_idx, valid, w_ce, n_kept = _route(x_flat, np.asarray(Wr))
    w1r, w2t, b1t = _reorder_weights(np.asarray(W1), np.asarray(W2), np.asarray(b1))

    # Slot 0 = the 8 lightest experts (processed first), slot 1 = heaviest.
    order = np.argsort(-n_kept, kind="stable")
    assign = [(int(order[E - 1 - i]), int(order[i])) for i in range(NCORES)]
    ts = (
        _r16(max(n_kept[a] for a, _ in assign)),
        _r16(max(n_kept[b] for _, b in assign)),
    )
    tot = sum(ts)

    nc = _get_nc(ts)
    in_maps = []
    for c in range(NCORES):
        exps = assign[c]
        # gather + transpose tokens for each slot: [128, KD * T]
        xparts = []
        for s, e in enumerate(exps):
            ids = top_idx[: n_kept[e], e]
            xg = np.zeros((ts[s], D), np.float32)
            xg[: len(ids)] = x_flat[ids]
            xparts.append(
                xg.reshape(ts[s], KD, 128).transpose(2, 1, 0).reshape(128, KD * ts[s])
            )
        xt_c = np.ascontiguousarray(
            np.concatenate(xparts, axis=1), dtype=ml_dtypes.bfloat16
        )
        el = list(exps)
        in_maps.append(
            {
                "xt": xt_c,
                "w1r": np.ascontiguousarray(w1r[el]),
                "w2t": np.ascontiguousarray(w2t[el]),
                "b1t": np.ascontiguousarray(b1t[el]),
            }
        )
    res = run_bass_kernel_spmd(nc, in_maps, list(range(NCORES)), trace=_trace)
    LAST = res

    # Combine: scatter gate-weighted expert outputs back to token order.
    y_flat = np.zeros((N, D), np.float32)
    b2 = np.asarray(b2)
    add_b2 = bool(np.any(b2))
    for c in range(NCORES):
        yc = res.results[c]["y"]  # [D, tot]
        for s, e in enumerate(assign[c]):
            n = int(n_kept[e])
            ids = top_idx[:n, e]
            off = ts[0] if s else 0
            w = w_ce[:n, e]
            y_flat[ids] = (yc[:, off : off + n] * w[None, :]).T
            if add_b2:
                y_flat[ids] += w[:, None] * b2[e]

    # Dense fallback for fully-dropped tokens (rare; none at typical loads).
    dropped = ~kept
    if np.any(dropped):
        xd = x_flat[dropped]
        hf = xd @ np.asarray(W1f) + np.asarray(b1f)
        gf = (hf[:, :H] / (1.0 + np.exp(-hf[:, :H]))) * hf[:, H:]
        y_flat[dropped] += FALLBACK_W * (gf @ np.asarray(W2f) + np.asarray(b2f))

    return y_flat.reshape(B, S, D)
